# revision 38
# baseline (speedup 1.0000x reference)
"""BayesianLinear forward on 8 Trainium2 NeuronCores.

y = x @ W^T + b with W = w_mu + softplus(w_rho) * eps_w,
                     b = b_mu + softplus(b_rho) * eps_b.

Sharding: column-parallel (output features / 8). Each core samples its
weight shard on-chip and computes y^T[o_shard, :] = W_shard @ x^T.

Datapath: bf16 matmuls (x cast host-side; W sampled on-chip to a
resident 4 MiB bf16 shard; ACT evaluates exp/ln in fp32 internally),
fp32 PSUM accumulation over all 32 k-tiles per (token-tile,
out-subtile) group, one bias-fused drain per group. Total HBM traffic:
52 MiB/core vs 96 MiB for the fp32 version, which moves the kernel
from the DMA roofline to the bf16 PE roofline (~216 ns per
128x128x512 matmul, 2.4 GHz warm).

Schedule: all input DMA rides the SP HWDGE ring in one hand-ordered
stream (a single ring still spreads across all 16 SDMA engines = full
HBM bandwidth, and keeping compute-engine waits off the ring's issuing
engine avoids head-of-line blocking). Phase 1 runs token tiles 0 and 1
chunk-major in lockstep with param streaming + half-chunk-granular
sampling, sized so param+x delivery (~7 us/chunk at ~340 GB/s) stays
just ahead of PE consumption (7.1 us/chunk). Token tiles 2..7 then run
k-contiguous at full PE rate with x prefetched ahead. Dep-free junk
matmuls during the initial DMA fill warm the PE's HAM clock gate
(1.2 -> 2.4 GHz) before the first real matmul. Bias params ride the
idle SWDGE ring.
"""

import numpy as np

# Problem shape (hardcoded per contest rules; kernel.py must be self-contained).
IN_F = 4096
OUT_F = 4096
N_TOK = 4096
N_CORES = 8
O_SHARD = OUT_F // N_CORES  # 512 output features per core

P = 128                     # SBUF partitions
KT = IN_F // P              # 32 contraction k-tiles
MS = O_SHARD // P           # 4 output-feature subtiles per core
N_TILE = 512                # moving-operand tile (fp32 PSUM bank limit)
NT = N_TOK // N_TILE        # 8 token tiles
CH = 4                      # k-tiles per param/sampling chunk
NCH = KT // CH              # 8 chunks
XH = 8                      # k-tiles per x piece (1 MiB DMAs)
NXH = KT // XH              # 4 x pieces per token tile
LOOKAHEAD = 5               # x pieces emitted ahead of the PE

_CACHE = {}


def _pin_act_table(bacc, mybir):
    """Keep Exp and Ln only in the one ACT table that has both, so the
    compiler never inserts per-op table reloads (2.7 us each, and they sit
    on the weight-sampling critical path)."""
    if getattr(bacc.get_activation_tables, "_pinned", False):
        return
    orig = bacc.get_activation_tables
    EXP = mybir.ActivationFunctionType.Exp
    LN = mybir.ActivationFunctionType.Ln

    def pinned(arch):
        tables = orig(arch)
        for name, funcs in tables.items():
            if name != "natural_log_exp_and_others":
                funcs.discard(EXP)
                funcs.discard(LN)
        return tables

    pinned._pinned = True
    bacc.get_activation_tables = pinned


def _build_nc():
    import concourse.bass as bass  # noqa: F401
    from concourse import bacc, mybir
    from concourse.tile import TileContext

    _pin_act_table(bacc, mybir)

    f32 = mybir.dt.float32
    bf16 = mybir.dt.bfloat16
    AF = mybir.ActivationFunctionType

    nc = bacc.Bacc("TRN2", target_bir_lowering=False, debug=False,
                   num_devices=N_CORES)

    # host-swizzled partition-major layouts: every DMA lands as >=8 KiB
    # contiguous runs per partition (line-rate descriptors)
    x_t = nc.dram_tensor("x_t", [NT, NXH, P, XH, N_TILE], bf16,
                         kind="ExternalInput")
    # fp8 in DRAM (exact for the BayesianLinear -3.0 rho init; e4m3
    # covers +-3.5 sigma of a randn rho at ~2.7% RMS, a <0.1% W effect);
    # the SWDGE DMA casts to bf16 in the DMA engine, so ACT never reads
    # fp8 (direct fp8 ACT input faults the device)
    fp8 = mybir.dt.float8e4
    rho_t = nc.dram_tensor("rho_t", [NCH, P, CH, O_SHARD], fp8,
                           kind="ExternalInput")
    # per chunk: [P, CH, 2, o_shard]; plane 0 = eps, 1 = mu
    em_t = nc.dram_tensor("em_t", [NCH, P, CH, 2, O_SHARD], bf16,
                          kind="ExternalInput")
    # biases pre-laid-out [P, MS] on the host: row p, col s = b[s*128+p]
    b_mu_t = nc.dram_tensor("b_mu_t", [P, MS], f32, kind="ExternalInput")
    b_rho_t = nc.dram_tensor("b_rho_t", [P, MS], f32, kind="ExternalInput")
    eps_b_t = nc.dram_tensor("eps_b_t", [P, MS], f32, kind="ExternalInput")
    y_t = nc.dram_tensor("y_t", [O_SHARD, N_TOK], f32, kind="ExternalOutput")

    with TileContext(nc) as tc:
        with (
            tc.tile_pool(name="wpool", bufs=1) as wpool,
            tc.tile_pool(name="rpool", bufs=3) as rpool,
            tc.tile_pool(name="empool", bufs=3) as empool,
            tc.tile_pool(name="spool", bufs=3) as spool,
            tc.tile_pool(name="bpool", bufs=1) as bpool,
            tc.tile_pool(name="xpool", bufs=8) as xpool,
            tc.tile_pool(name="opool", bufs=8) as opool,
            tc.tile_pool(name="psum", bufs=8, space="PSUM") as psum,
        ):
            # ---- bias vector: b = b_mu + softplus(b_rho) * eps_b ----
            bmu_sb = bpool.tile([P, MS], f32, tag="bmu")
            brho_sb = bpool.tile([P, MS], f32, tag="brho")
            beps_sb = bpool.tile([P, MS], f32, tag="beps")
            bvec = bpool.tile([P, MS], f32, tag="bvec")

            def compute_bias():
                nc.gpsimd.dma_start(brho_sb[:], b_rho_t[:, :])
                nc.gpsimd.dma_start(beps_sb[:], eps_b_t[:, :])
                nc.gpsimd.dma_start(bmu_sb[:], b_mu_t[:, :])
                # softplus(r) = ln(1 + exp(r)); Exp/Ln share one ACT table.
                nc.scalar.activation(bvec[:], brho_sb[:], AF.Exp)
                nc.scalar.activation(bvec[:], bvec[:], AF.Ln, bias=1.0)
                nc.vector.tensor_mul(bvec[:], bvec[:], beps_sb[:])
                nc.vector.tensor_add(bvec[:], bvec[:], bmu_sb[:])

            # W shard, fully resident in bf16 (32 KiB/partition).
            w_sb = wpool.tile([P, KT, O_SHARD], bf16, tag="w")

            def load_rho(c):
                r = rpool.tile([P, CH, O_SHARD], bf16, tag="rho",
                               name=f"rho_{c}")
                # SWDGE ring (cast fp8->bf16 in the DMA engine); this also
                # takes the rho stream off the saturated SP ring, and the
                # 0.25 MiB chunks are paced by rho-slot recycling so they
                # never flood HBM during the critical head window
                if c == 0:
                    # halves: exp on k-tiles 0-1 starts one transfer sooner
                    nc.gpsimd.dma_start(r[:, 0:2, :], rho_t[c, :, 0:2, :])
                    nc.gpsimd.dma_start(r[:, 2:CH, :], rho_t[c, :, 2:CH, :])
                else:
                    nc.gpsimd.dma_start(r[:], rho_t[c, :, :, :])
                return r

            def load_em(c):
                em = empool.tile([P, CH, 2, O_SHARD], bf16, tag="em",
                                 name=f"em_{c}")
                nc.sync.dma_start(em[:], em_t[c, :, :, :, :])
                return em

            def sample(c, rho, em, lo, hi):
                # sigma = ln(1 + exp(rho)) staged in bf16 (ACT computes in
                # fp32 internally; bf16 storage of exp(rho) costs <0.04%
                # on sigma); W slice = sigma * eps + mu. Half-chunk (2
                # k-tile) granularity halves the W-ready latency and lets
                # chunk matmuls start on k-tiles 0-1 while 2-3 sample.
                s = spool.tile([P, CH, O_SHARD], bf16, tag="s",
                               name=f"s_{c}_{lo}")
                nc.scalar.activation(s[:, lo:hi, :], rho[:, lo:hi, :],
                                     AF.Exp)
                nc.scalar.activation(s[:, lo:hi, :], s[:, lo:hi, :],
                                     AF.Ln, bias=1.0)
                nc.vector.tensor_mul(s[:, lo:hi, :], s[:, lo:hi, :],
                                     em[:, lo:hi, 0, :])
                nc.vector.tensor_add(w_sb[:, c * CH + lo:c * CH + hi, :],
                                     s[:, lo:hi, :], em[:, lo:hi, 1, :])

            # x^T pieces [P, XH, N_TILE] bf16 (1 MiB straight copies).
            x_tiles = {}

            def alloc_x(nt, h):
                xt = xpool.tile([P, XH, N_TILE], bf16, tag="x",
                                name=f"xt_{nt}_{h}")
                x_tiles[(nt, h)] = xt
                return xt

            def emit_x(nt, h, half=None, eng=None):
                eng = eng or nc.sync
                xt = x_tiles.get((nt, h)) or alloc_x(nt, h)
                if half is None:
                    eng.dma_start(xt[:], x_t[nt, h, :, :, :])
                elif half == 0:
                    eng.dma_start(xt[:, 0:CH, :], x_t[nt, h, :, 0:CH, :])
                else:
                    eng.dma_start(xt[:, CH:XH, :], x_t[nt, h, :, CH:XH, :])

            def mm_chunk(ps, nt, c):
                xt = x_tiles[(nt, c // 2)]
                for j in range(CH):
                    kt = c * CH + j
                    for ms in range(MS):
                        nc.tensor.matmul(
                            ps[ms][:],
                            lhsT=w_sb[:, kt, ms * P:(ms + 1) * P],
                            rhs=xt[:, (c % 2) * CH + j, :],
                            start=(kt == 0),
                            stop=(kt == KT - 1),
                        )

            def drain(ps, nt):
                # split PSUM->SBUF bias-fused drains across DVE and ACT so
                # each group's evacuation takes ~2 op-times, not 4.
                nsl = slice(nt * N_TILE, (nt + 1) * N_TILE)
                for ms in range(MS):
                    ot = opool.tile([P, N_TILE], f32, tag="o",
                                    name=f"of_{nt}_{ms}")
                    if ms < 2:
                        nc.vector.tensor_scalar_add(ot[:], ps[ms][:],
                                                    bvec[:, ms:ms + 1])
                    else:
                        nc.scalar.activation(ot[:], ps[ms][:], AF.Identity,
                                             bias=bvec[:, ms:ms + 1])
                    nc.gpsimd.dma_start(y_t[ms * P:(ms + 1) * P, nsl], ot[:])

            def psum_group(nt):
                return [psum.tile([P, N_TILE], f32, tag="ps",
                                  name=f"ps_{nt}_{ms}")
                        for ms in range(MS)]

            ps01 = {nt: psum_group(nt) for nt in (0, 1)}

            # HAM warm-up: dep-free junk matmuls fill the otherwise-idle
            # PE during the first param DMA, so the 4096-cycle activity
            # window un-throttles the clock (1.2 -> 2.4 GHz) before the
            # first real matmul instead of ~3.4 us into the stream. The
            # real start=True matmuls clear the banks afterwards.
            junk = bpool.tile([P, N_TILE], bf16, tag="junk")
            nc.gpsimd.memset(junk[:], 0)
            for i in range(24):
                nc.tensor.matmul(ps01[0][i % 4][:], lhsT=junk[:, 0:P],
                                 rhs=junk[:], start=True, stop=True)

            # Phase 1: token tiles 0 and 1 chunk-major, in lockstep with
            # param streaming + sampling. The SP ring carries, in order:
            #   rho0 x00a em0 x10a | rho1 em1 x00b x10b | rho2 em2 x01 |
            #   rho3 em3 x11 | ... | rho7 em7 x13 | x20 x21
            # so delivery (~6.4 us/chunk at ~340 GB/s) stays one chunk
            # ahead of PE consumption (7.1 us/chunk) throughout.
            x_slot = {2: [(0, 1), (1, 1)], 4: [(0, 2), (1, 2)],
                      6: [(0, 3), (1, 3)]}
            rho0 = load_rho(0)
            emit_x(0, 0, half=0)
            em0 = load_em(0)
            emit_x(1, 0, half=0)
            parts = {0: (rho0, em0)}
            for c in range(NCH):
                if c >= 1:
                    parts[c] = (load_rho(c), load_em(c))
                    if c == 1:
                        emit_x(0, 0, half=1)
                        emit_x(1, 0, half=1)
                    for p in x_slot.get(c, []):
                        emit_x(*p)
                rho, em = parts[c]
                if c == 0:
                    # 1-k-tile first piece: shortest possible chain to the
                    # first real matmul
                    sample(c, rho, em, 0, 1)
                    sample(c, rho, em, 1, 2)
                else:
                    sample(c, rho, em, 0, 2)
                sample(c, rho, em, 2, CH)
                if c == 0:
                    # bias compute off the critical sampling head; only
                    # needed by the first drain ~60 us in
                    compute_bias()
                if c == NCH - 1:
                    # phase-2 head start on the ring tail
                    emit_x(2, 0)
                    emit_x(2, 1)
                for nt in (0, 1):
                    mm_chunk(ps01[nt], nt, c)
            for nt in (0, 1):
                drain(ps01[nt], nt)

            # Phase 2: token tiles 2..7 k-contiguous at full PE rate; x
            # pieces emitted LOOKAHEAD ahead of consumption.
            x_order = [(nt, h) for nt in range(2, NT) for h in range(NXH)]
            emitted = [2]  # (2,0) and (2,1) already on the ring

            def emit_phase2_upto(i):
                while emitted[0] <= min(i, len(x_order) - 1):
                    emit_x(*x_order[emitted[0]])
                    emitted[0] += 1

            for nt in range(2, NT):
                ps = psum_group(nt)
                for c in range(NCH):
                    emit_phase2_upto(x_order.index((nt, c // 2)) + LOOKAHEAD)
                    mm_chunk(ps, nt, c)
                drain(ps, nt)

    nc.compile()
    return nc


def _get_nc():
    if "nc" not in _CACHE:
        _CACHE["nc"] = _build_nc()
    return _CACHE["nc"]


def _in_maps(inputs):
    import ml_dtypes

    bf16 = ml_dtypes.bfloat16
    x = np.asarray(inputs["x"], dtype=np.float32)
    w_mu = np.asarray(inputs["w_mu"], dtype=np.float32)
    w_rho = np.asarray(inputs["w_rho"], dtype=np.float32)
    eps_w = np.asarray(inputs["eps_w"], dtype=np.float32)
    b_mu = np.asarray(inputs["b_mu"], dtype=np.float32)
    b_rho = np.asarray(inputs["b_rho"], dtype=np.float32)
    eps_b = np.asarray(inputs["eps_b"], dtype=np.float32)

    # x_t[nt, h, p, j, n] = x.T[h*XH*P + j*P + p, nt*N_TILE + n]
    x_t = np.ascontiguousarray(
        x.T.astype(bf16).reshape(NXH, XH, P, NT, N_TILE)
        .transpose(3, 0, 2, 1, 4))
    maps = []
    for c in range(N_CORES):
        sl = slice(c * O_SHARD, (c + 1) * O_SHARD)
        # rho_t[c, p, j, o] = w_rho.T[c*CH*P + j*P + p, o]
        rho_sw = np.ascontiguousarray(
            w_rho[sl].T.astype(ml_dtypes.float8_e4m3fn)
            .reshape(NCH, CH, P, O_SHARD).transpose(0, 2, 1, 3))
        em = np.stack([eps_w[sl].T, w_mu[sl].T],
                      axis=1).astype(bf16)  # [IN_F, 2, O_SHARD]
        # em_t[c, p, j, t, o] = em[c*CH*P + j*P + p, t, o]
        em_sw = np.ascontiguousarray(
            em.reshape(NCH, CH, P, 2, O_SHARD).transpose(0, 2, 1, 3, 4))
        maps.append({
            "x_t": x_t,
            "rho_t": rho_sw,
            "em_t": em_sw,
            "b_mu_t": np.ascontiguousarray(b_mu[sl].reshape(MS, P).T),
            "b_rho_t": np.ascontiguousarray(b_rho[sl].reshape(MS, P).T),
            "eps_b_t": np.ascontiguousarray(eps_b[sl].reshape(MS, P).T),
        })
    return maps


def run(inputs, trace=False, **kwargs):
    """Run on hardware; returns (y [N_TOK, OUT_F], BassKernelResults)."""
    from concourse.bass_utils import run_bass_kernel_spmd

    nc = _get_nc()
    res = run_bass_kernel_spmd(nc, _in_maps(inputs), list(range(N_CORES)),
                               trace=trace, **kwargs)
    y_t = np.concatenate([r["y_t"] for r in res.results], axis=0)
    return np.ascontiguousarray(y_t.T), res


def kernel(**inputs) -> np.ndarray:
    y, _ = run(inputs, trace=False)
    return y


# revision 39
# speedup vs baseline: 1.0213x; 1.0213x over previous
"""BayesianLinear forward on 8 Trainium2 NeuronCores.

y = x @ W^T + b with W = w_mu + softplus(w_rho) * eps_w,
                     b = b_mu + softplus(b_rho) * eps_b.

Sharding: column-parallel (output features / 8). Each core samples its
weight shard on-chip and computes y^T[o_shard, :] = W_shard @ x^T.

Datapath: bf16 matmuls (x cast host-side; W sampled on-chip to a
resident 4 MiB bf16 shard; ACT evaluates exp/ln in fp32 internally),
fp32 PSUM accumulation over all 32 k-tiles per (token-tile,
out-subtile) group, one bias-fused drain per group. Total HBM traffic:
52 MiB/core vs 96 MiB for the fp32 version, which moves the kernel
from the DMA roofline to the bf16 PE roofline (~216 ns per
128x128x512 matmul, 2.4 GHz warm).

Schedule: all input DMA rides the SP HWDGE ring in one hand-ordered
stream (a single ring still spreads across all 16 SDMA engines = full
HBM bandwidth, and keeping compute-engine waits off the ring's issuing
engine avoids head-of-line blocking). Phase 1 runs token tiles 0 and 1
chunk-major in lockstep with param streaming + half-chunk-granular
sampling, sized so param+x delivery (~7 us/chunk at ~340 GB/s) stays
just ahead of PE consumption (7.1 us/chunk). Token tiles 2..7 then run
k-contiguous at full PE rate with x prefetched ahead. Dep-free junk
matmuls during the initial DMA fill warm the PE's HAM clock gate
(1.2 -> 2.4 GHz) before the first real matmul. Bias params ride the
idle SWDGE ring.
"""

import numpy as np

# Problem shape (hardcoded per contest rules; kernel.py must be self-contained).
IN_F = 4096
OUT_F = 4096
N_TOK = 4096
N_CORES = 8
O_SHARD = OUT_F // N_CORES  # 512 output features per core

P = 128                     # SBUF partitions
KT = IN_F // P              # 32 contraction k-tiles
MS = O_SHARD // P           # 4 output-feature subtiles per core
N_TILE = 512                # moving-operand tile (fp32 PSUM bank limit)
NT = N_TOK // N_TILE        # 8 token tiles
CH = 4                      # k-tiles per param/sampling chunk
NCH = KT // CH              # 8 chunks
XH = 8                      # k-tiles per x piece (1 MiB DMAs)
NXH = KT // XH              # 4 x pieces per token tile
LOOKAHEAD = 5               # x pieces emitted ahead of the PE

_CACHE = {}


def _pin_act_table(bacc, mybir):
    """Keep Exp and Ln only in the one ACT table that has both, so the
    compiler never inserts per-op table reloads (2.7 us each, and they sit
    on the weight-sampling critical path)."""
    if getattr(bacc.get_activation_tables, "_pinned", False):
        return
    orig = bacc.get_activation_tables
    EXP = mybir.ActivationFunctionType.Exp
    LN = mybir.ActivationFunctionType.Ln

    def pinned(arch):
        tables = orig(arch)
        for name, funcs in tables.items():
            if name != "natural_log_exp_and_others":
                funcs.discard(EXP)
                funcs.discard(LN)
        return tables

    pinned._pinned = True
    bacc.get_activation_tables = pinned


def _build_nc():
    import concourse.bass as bass  # noqa: F401
    from concourse import bacc, mybir
    from concourse.tile import TileContext

    _pin_act_table(bacc, mybir)

    f32 = mybir.dt.float32
    bf16 = mybir.dt.bfloat16
    AF = mybir.ActivationFunctionType

    nc = bacc.Bacc("TRN2", target_bir_lowering=False, debug=False,
                   num_devices=N_CORES)

    # host-swizzled partition-major layouts: every DMA lands as >=8 KiB
    # contiguous runs per partition (line-rate descriptors)
    x_t = nc.dram_tensor("x_t", [NT, NXH, P, XH, N_TILE], bf16,
                         kind="ExternalInput")
    rho_t = nc.dram_tensor("rho_t", [NCH, P, CH, O_SHARD], bf16,
                           kind="ExternalInput")
    # per chunk: [P, CH, 2, o_shard]; plane 0 = eps, 1 = mu
    em_t = nc.dram_tensor("em_t", [NCH, P, CH, 2, O_SHARD], bf16,
                          kind="ExternalInput")
    # biases pre-laid-out [P, MS] on the host: row p, col s = b[s*128+p]
    b_mu_t = nc.dram_tensor("b_mu_t", [P, MS], f32, kind="ExternalInput")
    b_rho_t = nc.dram_tensor("b_rho_t", [P, MS], f32, kind="ExternalInput")
    eps_b_t = nc.dram_tensor("eps_b_t", [P, MS], f32, kind="ExternalInput")
    y_t = nc.dram_tensor("y_t", [O_SHARD, N_TOK], f32, kind="ExternalOutput")

    with TileContext(nc) as tc:
        with (
            tc.tile_pool(name="wpool", bufs=1) as wpool,
            tc.tile_pool(name="rpool", bufs=3) as rpool,
            tc.tile_pool(name="empool", bufs=3) as empool,
            tc.tile_pool(name="spool", bufs=3) as spool,
            tc.tile_pool(name="bpool", bufs=1) as bpool,
            tc.tile_pool(name="xpool", bufs=8) as xpool,
            tc.tile_pool(name="opool", bufs=8) as opool,
            tc.tile_pool(name="psum", bufs=8, space="PSUM") as psum,
        ):
            # ---- bias vector: b = b_mu + softplus(b_rho) * eps_b ----
            bmu_sb = bpool.tile([P, MS], f32, tag="bmu")
            brho_sb = bpool.tile([P, MS], f32, tag="brho")
            beps_sb = bpool.tile([P, MS], f32, tag="beps")
            bvec = bpool.tile([P, MS], f32, tag="bvec")

            def compute_bias():
                nc.gpsimd.dma_start(brho_sb[:], b_rho_t[:, :])
                nc.gpsimd.dma_start(beps_sb[:], eps_b_t[:, :])
                nc.gpsimd.dma_start(bmu_sb[:], b_mu_t[:, :])
                # softplus(r) = ln(1 + exp(r)); Exp/Ln share one ACT table.
                nc.scalar.activation(bvec[:], brho_sb[:], AF.Exp)
                nc.scalar.activation(bvec[:], bvec[:], AF.Ln, bias=1.0)
                nc.vector.tensor_mul(bvec[:], bvec[:], beps_sb[:])
                nc.vector.tensor_add(bvec[:], bvec[:], bmu_sb[:])

            # W shard, fully resident in bf16 (32 KiB/partition).
            w_sb = wpool.tile([P, KT, O_SHARD], bf16, tag="w")

            def load_rho(c):
                r = rpool.tile([P, CH, O_SHARD], bf16, tag="rho",
                               name=f"rho_{c}")
                if c == 0:
                    # halves: exp on k-tiles 0-1 starts one transfer sooner
                    nc.sync.dma_start(r[:, 0:2, :], rho_t[c, :, 0:2, :])
                    nc.sync.dma_start(r[:, 2:CH, :], rho_t[c, :, 2:CH, :])
                else:
                    nc.sync.dma_start(r[:], rho_t[c, :, :, :])
                return r

            def load_em(c):
                em = empool.tile([P, CH, 2, O_SHARD], bf16, tag="em",
                                 name=f"em_{c}")
                nc.sync.dma_start(em[:], em_t[c, :, :, :, :])
                return em

            def sample(c, rho, em, lo, hi):
                # sigma = ln(1 + exp(rho)) staged in bf16 (ACT computes in
                # fp32 internally; bf16 storage of exp(rho) costs <0.04%
                # on sigma); W slice = sigma * eps + mu. Half-chunk (2
                # k-tile) granularity halves the W-ready latency and lets
                # chunk matmuls start on k-tiles 0-1 while 2-3 sample.
                s = spool.tile([P, CH, O_SHARD], bf16, tag="s",
                               name=f"s_{c}_{lo}")
                nc.scalar.activation(s[:, lo:hi, :], rho[:, lo:hi, :],
                                     AF.Exp)
                nc.scalar.activation(s[:, lo:hi, :], s[:, lo:hi, :],
                                     AF.Ln, bias=1.0)
                nc.vector.tensor_mul(s[:, lo:hi, :], s[:, lo:hi, :],
                                     em[:, lo:hi, 0, :])
                nc.vector.tensor_add(w_sb[:, c * CH + lo:c * CH + hi, :],
                                     s[:, lo:hi, :], em[:, lo:hi, 1, :])

            # x^T pieces [P, XH, N_TILE] bf16 (1 MiB straight copies).
            x_tiles = {}

            def alloc_x(nt, h):
                xt = xpool.tile([P, XH, N_TILE], bf16, tag="x",
                                name=f"xt_{nt}_{h}")
                x_tiles[(nt, h)] = xt
                return xt

            def emit_x(nt, h, half=None, eng=None):
                eng = eng or nc.sync
                xt = x_tiles.get((nt, h)) or alloc_x(nt, h)
                if half is None:
                    eng.dma_start(xt[:], x_t[nt, h, :, :, :])
                elif half == 0:
                    eng.dma_start(xt[:, 0:CH, :], x_t[nt, h, :, 0:CH, :])
                else:
                    eng.dma_start(xt[:, CH:XH, :], x_t[nt, h, :, CH:XH, :])

            def mm_chunk(ps, nt, c):
                xt = x_tiles[(nt, c // 2)]
                for j in range(CH):
                    kt = c * CH + j
                    for ms in range(MS):
                        nc.tensor.matmul(
                            ps[ms][:],
                            lhsT=w_sb[:, kt, ms * P:(ms + 1) * P],
                            rhs=xt[:, (c % 2) * CH + j, :],
                            start=(kt == 0),
                            stop=(kt == KT - 1),
                        )

            def drain(ps, nt):
                # split PSUM->SBUF bias-fused drains across DVE and ACT so
                # each group's evacuation takes ~2 op-times, not 4.
                nsl = slice(nt * N_TILE, (nt + 1) * N_TILE)
                for ms in range(MS):
                    ot = opool.tile([P, N_TILE], f32, tag="o",
                                    name=f"of_{nt}_{ms}")
                    if ms < 2:
                        nc.vector.tensor_scalar_add(ot[:], ps[ms][:],
                                                    bvec[:, ms:ms + 1])
                    else:
                        nc.scalar.activation(ot[:], ps[ms][:], AF.Identity,
                                             bias=bvec[:, ms:ms + 1])
                    nc.gpsimd.dma_start(y_t[ms * P:(ms + 1) * P, nsl], ot[:])

            def psum_group(nt):
                return [psum.tile([P, N_TILE], f32, tag="ps",
                                  name=f"ps_{nt}_{ms}")
                        for ms in range(MS)]

            ps01 = {nt: psum_group(nt) for nt in (0, 1)}

            # HAM warm-up: dep-free junk matmuls fill the otherwise-idle
            # PE during the first param DMA, so the 4096-cycle activity
            # window un-throttles the clock (1.2 -> 2.4 GHz) before the
            # first real matmul instead of ~3.4 us into the stream. The
            # real start=True matmuls clear the banks afterwards.
            junk = bpool.tile([P, N_TILE], bf16, tag="junk")
            nc.gpsimd.memset(junk[:], 0)
            for i in range(24):
                nc.tensor.matmul(ps01[0][i % 4][:], lhsT=junk[:, 0:P],
                                 rhs=junk[:], start=True, stop=True)

            # Phase 1: token tiles 0 and 1 chunk-major, in lockstep with
            # param streaming + sampling. The SP ring carries, in order:
            #   rho0 x00a em0 x10a | rho1 em1 x00b x10b | rho2 em2 x01 |
            #   rho3 em3 x11 | ... | rho7 em7 x13 | x20 x21
            # so delivery (~6.4 us/chunk at ~340 GB/s) stays one chunk
            # ahead of PE consumption (7.1 us/chunk) throughout.
            x_slot = {2: [(0, 1), (1, 1)], 4: [(0, 2), (1, 2)],
                      6: [(0, 3), (1, 3)]}
            rho0 = load_rho(0)
            emit_x(0, 0, half=0)
            em0 = load_em(0)
            emit_x(1, 0, half=0)
            parts = {0: (rho0, em0)}
            for c in range(NCH):
                if c >= 1:
                    parts[c] = (load_rho(c), load_em(c))
                    if c == 1:
                        emit_x(0, 0, half=1)
                        emit_x(1, 0, half=1)
                    for p in x_slot.get(c, []):
                        emit_x(*p)
                rho, em = parts[c]
                if c == 0:
                    # 1-k-tile first piece: shortest possible chain to the
                    # first real matmul
                    sample(c, rho, em, 0, 1)
                    sample(c, rho, em, 1, 2)
                else:
                    sample(c, rho, em, 0, 2)
                sample(c, rho, em, 2, CH)
                if c == 0:
                    # bias compute off the critical sampling head; only
                    # needed by the first drain ~60 us in
                    compute_bias()
                if c == NCH - 1:
                    # phase-2 head start on the ring tail
                    emit_x(2, 0)
                    emit_x(2, 1)
                for nt in (0, 1):
                    mm_chunk(ps01[nt], nt, c)
            for nt in (0, 1):
                drain(ps01[nt], nt)

            # Phase 2: token tiles 2..7 k-contiguous at full PE rate; x
            # pieces emitted LOOKAHEAD ahead of consumption.
            x_order = [(nt, h) for nt in range(2, NT) for h in range(NXH)]
            emitted = [2]  # (2,0) and (2,1) already on the ring

            def emit_phase2_upto(i):
                while emitted[0] <= min(i, len(x_order) - 1):
                    emit_x(*x_order[emitted[0]])
                    emitted[0] += 1

            for nt in range(2, NT):
                ps = psum_group(nt)
                for c in range(NCH):
                    emit_phase2_upto(x_order.index((nt, c // 2)) + LOOKAHEAD)
                    mm_chunk(ps, nt, c)
                drain(ps, nt)

    nc.compile()
    return nc


def _get_nc():
    if "nc" not in _CACHE:
        _CACHE["nc"] = _build_nc()
    return _CACHE["nc"]


def _in_maps(inputs):
    import ml_dtypes

    bf16 = ml_dtypes.bfloat16
    x = np.asarray(inputs["x"], dtype=np.float32)
    w_mu = np.asarray(inputs["w_mu"], dtype=np.float32)
    w_rho = np.asarray(inputs["w_rho"], dtype=np.float32)
    eps_w = np.asarray(inputs["eps_w"], dtype=np.float32)
    b_mu = np.asarray(inputs["b_mu"], dtype=np.float32)
    b_rho = np.asarray(inputs["b_rho"], dtype=np.float32)
    eps_b = np.asarray(inputs["eps_b"], dtype=np.float32)

    # x_t[nt, h, p, j, n] = x.T[h*XH*P + j*P + p, nt*N_TILE + n]
    x_t = np.ascontiguousarray(
        x.T.astype(bf16).reshape(NXH, XH, P, NT, N_TILE)
        .transpose(3, 0, 2, 1, 4))
    maps = []
    for c in range(N_CORES):
        sl = slice(c * O_SHARD, (c + 1) * O_SHARD)
        # rho_t[c, p, j, o] = w_rho.T[c*CH*P + j*P + p, o]
        rho_sw = np.ascontiguousarray(
            w_rho[sl].T.astype(bf16).reshape(NCH, CH, P, O_SHARD)
            .transpose(0, 2, 1, 3))
        em = np.stack([eps_w[sl].T, w_mu[sl].T],
                      axis=1).astype(bf16)  # [IN_F, 2, O_SHARD]
        # em_t[c, p, j, t, o] = em[c*CH*P + j*P + p, t, o]
        em_sw = np.ascontiguousarray(
            em.reshape(NCH, CH, P, 2, O_SHARD).transpose(0, 2, 1, 3, 4))
        maps.append({
            "x_t": x_t,
            "rho_t": rho_sw,
            "em_t": em_sw,
            "b_mu_t": np.ascontiguousarray(b_mu[sl].reshape(MS, P).T),
            "b_rho_t": np.ascontiguousarray(b_rho[sl].reshape(MS, P).T),
            "eps_b_t": np.ascontiguousarray(eps_b[sl].reshape(MS, P).T),
        })
    return maps


def run(inputs, trace=False, **kwargs):
    """Run on hardware; returns (y [N_TOK, OUT_F], BassKernelResults)."""
    from concourse.bass_utils import run_bass_kernel_spmd

    nc = _get_nc()
    res = run_bass_kernel_spmd(nc, _in_maps(inputs), list(range(N_CORES)),
                               trace=trace, **kwargs)
    y_t = np.concatenate([r["y_t"] for r in res.results], axis=0)
    return np.ascontiguousarray(y_t.T), res


def kernel(**inputs) -> np.ndarray:
    y, _ = run(inputs, trace=False)
    return y
